# revision 11
# baseline (speedup 1.0000x reference)
# Involution2d (K=7) Trainium2 kernel — 8-core SPMD, batch+spatial sharding.
#
# V3: the multiply-accumulate runs on TensorE as diag-matmuls.
# For each 128-pixel block (2 image rows) the involution is
#   out_T[q, c] = sum_o ker[o, P0+q] * xT[P0 + q + s_o, c]
#               = sum_o matmul(lhsT=diag(kerT_blk[:, o]), rhs=xT_shift_o_blk)
# accumulated across the 49 offsets in fp32 PSUM.  Two tricks make every
# operand a plain aligned tile:
#   - the host ships 14 shifted transposed-x copies (7 column shifts dj x 2
#     row parities); the remaining row shift is then an even multiple of 64
#     = a whole number of 128-pixel blocks, i.e. pure tile indexing.
#   - the diag stationaries use unshifted kerT columns: one fp16 4x-mode
#     tensor_scalar (identity * per-partition scalar) per (offset, block),
#     split between VectorE and ScalarE.
# kerT ([pixel, 49] layout) is generated directly by making the f-slice the
# matmul stationary; b2 rides an appended ones-row, the W-edge mask (which
# also kills every row-wrap artifact of pixel-index shifting) multiplies in
# PSUM.  V2 (145us) was Vector-bound on broadcast-copy/mul/add chains; V3
# replaces all of that with 784 small matmuls + 784 tiny diag builds.
import numpy as np

EPS = 1e-5
KK = 7
C = 128
H = 64
W = 64
B = 4
HH = 32              # rows per core
P = HH * W           # 2048 output pixels per core
NBLK = 16            # 128-pixel blocks per core
XBLK = 19            # blocks per shifted-x copy (offsets -2..+1 around 16)
GEN_CHUNK = 512

_STATE = {}


def _build():
    import concourse.tile as tile
    from concourse import bacc, mybir

    f32 = mybir.dt.float32
    f16 = mybir.dt.float16
    nc = bacc.Bacc("TRN2", target_bir_lowering=False, debug=False)

    # xT14[k, (dj*2+par)*XBLK*128 + blk*128 + c]
    xt_d = nc.dram_tensor("xt14", [C, 14 * XBLK * 128], f16, kind="ExternalInput").ap()
    xn_d = nc.dram_tensor("xn", [C, P], f16, kind="ExternalInput").ap()
    w1sT_d = nc.dram_tensor("w1sT", [C, 32], f16, kind="ExternalInput").ap()
    b1f_d = nc.dram_tensor("b1f", [32, 1], f32, kind="ExternalInput").ap()
    w2Ta_d = nc.dram_tensor("w2Ta", [33, 49], f16, kind="ExternalInput").ap()
    maskT_d = nc.dram_tensor("maskT", [C, NBLK * 49], f16, kind="ExternalInput").ap()
    eye_d = nc.dram_tensor("eyec", [C, 64], f16, kind="ExternalInput").ap()
    out_d = nc.dram_tensor("out", [C, P], f16, kind="ExternalOutput").ap()

    with tile.TileContext(nc) as tc:
        with (
            tc.tile_pool(name="consts", bufs=1) as cpool,
            tc.tile_pool(name="dpool", bufs=1) as dpool,
            tc.tile_pool(name="pgen", bufs=1, space="PSUM") as pgen,
            tc.tile_pool(name="pout", bufs=1, space="PSUM") as pout,
        ):
            xn = cpool.tile([C, P], f16, tag="xn")
            nc.sync.dma_start(xn[:], xn_d)
            w1sT = cpool.tile([C, 32], f16, tag="w1")
            nc.sync.dma_start(w1sT[:], w1sT_d)
            b1f = cpool.tile([32, 1], f32, tag="b1")
            nc.sync.dma_start(b1f[:], b1f_d)
            w2Ta = cpool.tile([33, 49], f16, tag="w2")
            nc.sync.dma_start(w2Ta[:], w2Ta_d)
            maskT = cpool.tile([C, NBLK * 49], f16, tag="mask")
            nc.sync.dma_start(maskT[:], maskT_d)
            eye = cpool.tile([C, 64], f16, tag="eye")
            nc.sync.dma_start(eye[:], eye_d)
            xtc = []
            for cpy in range(14):
                t = cpool.tile([C, XBLK * 128], f16, tag=f"xtc{cpy}")
                nc.sync.dma_start(
                    t[:], xt_d[:, cpy * XBLK * 128:(cpy + 1) * XBLK * 128]
                )
                xtc.append(t)

            f_aug = cpool.tile([33, P], f16, tag="f")
            nc.vector.memset(f_aug[32:33, :], 1.0)
            kerT = cpool.tile([C, NBLK * 49], f32, tag="kerT")
            outT = cpool.tile([C, P], f16, tag="outT")

            # ---- kernel generation: f = relu(w1s^T x + b1) ----
            for ci in range(P // GEN_CHUNK):
                sl = slice(ci * GEN_CHUNK, (ci + 1) * GEN_CHUNK)
                f1 = pgen.tile([32, GEN_CHUNK], f32, tag="f1")
                nc.tensor.matmul(f1[:], w1sT[:], xn[:, sl], start=True, stop=True)
                nc.scalar.activation(
                    f_aug[0:32, sl], f1[:], mybir.ActivationFunctionType.Relu,
                    bias=b1f[:],
                )

            # ---- kerT[q, o] per block: (f_blk^T w2 + b2) * maskT ----
            for blk in range(NBLK):
                ksl = slice(blk * 49, (blk + 1) * 49)
                kps = pgen.tile([C, 512], f32, tag="kps")
                nc.tensor.matmul(
                    kps[:, 0:49], f_aug[:, blk * 128:(blk + 1) * 128], w2Ta[:],
                    start=True, stop=True,
                )
                nc.vector.tensor_mul(kerT[:, ksl], kps[:, 0:49], maskT[:, ksl])

            # ---- involution: per block, one batched diag build (compact
            # [k, o, j] sub-diags) + 49 x 4 concurrent 32x32 sub-matmuls ----
            NSE = 11       # diag builds done one-by-one on ScalarE
            NDV = 49 - NSE  # diag builds batched on VectorE

            def cpy_of(o):
                di, dj = divmod(o, 7)
                return dj * 2 + ((di - 3) & 1)

            order = sorted(range(49), key=cpy_of)
            for grp in ([0, 1, 2, 3, 4, 5], [6, 7, 8, 9, 10, 11],
                        [12, 13, 14, 15]):
                opsd, dald = {}, {}
                for blk in grp:
                    slot = blk % 6
                    ops_t = pout.tile([C, 512], f32, tag=f"ops{slot}")
                    opsd[blk] = ops_t
                    d_all = dpool.tile([C, 49 * 64], f16, tag=f"dall{slot}")
                    dald[blk] = d_all
                    nc.vector.tensor_mul(
                        d_all[:, 0:NDV * 64].rearrange("k (o j) -> k o j", j=64),
                        eye[:].unsqueeze(1).broadcast_to([C, NDV, 64]),
                        kerT[:, blk * 49: blk * 49 + NDV].unsqueeze(2)
                            .broadcast_to([C, NDV, 64]),
                    )
                    for o in range(NDV, 49):
                        nc.scalar.activation(
                            d_all[:, o * 64:(o + 1) * 64], eye[:],
                            mybir.ActivationFunctionType.Copy,
                            scale=kerT[:, blk * 49 + o: blk * 49 + o + 1],
                        )
                for idx, o in enumerate(order):
                    di, dj = divmod(o, 7)
                    sh = di - 3
                    par = sh & 1
                    m = (sh - par) >> 1           # in {-2, -1, 0, 1}
                    cpy = dj * 2 + par
                    for blk in grp:
                        xoff = (blk + m + 2) * 128
                        for i in range(2):
                            nc.tensor.matmul(
                                opsd[blk][64 * i:64 * i + 64, 0:128],
                                dald[blk][64 * i:64 * i + 64,
                                          o * 64:(o + 1) * 64],
                                xtc[cpy][64 * i:64 * i + 64,
                                         xoff: xoff + 128],
                                start=(idx == 0), stop=(idx == 48),
                                tile_position=(64 * i, 64 * i),
                            )
                for blk in grp:
                    nc.vector.tensor_copy(
                        outT[:, blk * 128:(blk + 1) * 128],
                        opsd[blk][:, 0:128],
                    )

            nc.sync.dma_start(out_d, outT[:])

    nc.compile()
    return nc


def _get_nc():
    if "nc" not in _STATE:
        _STATE["nc"] = _build()
    return _STATE["nc"]


def _host_prep(x, w1, b1, bn_gamma, bn_beta, bn_mean, bn_var, w2, b2):
    x = np.asarray(x, dtype=np.float32)
    scale = np.asarray(bn_gamma) / np.sqrt(np.asarray(bn_var) + EPS)
    w1s = (np.asarray(w1) * scale[:, None]).astype(np.float32)
    b1f = (np.asarray(b1) * scale + np.asarray(bn_beta)
           - np.asarray(bn_mean) * scale).astype(np.float32)
    w1sT = np.ascontiguousarray(w1s.T.astype(np.float16))        # [128, 32]
    b1fc = np.ascontiguousarray(b1f[:, None].astype(np.float32))
    w2Ta = np.zeros((33, 49), dtype=np.float16)
    w2Ta[:32] = np.asarray(w2, np.float32).T.astype(np.float16)
    w2Ta[32] = np.asarray(b2, np.float32).astype(np.float16)

    # maskT[k, blk*49 + o]: kernel zeroed where w + dj leaves the row
    wcol = np.arange(P, dtype=np.int64) % W
    mask = np.zeros((49, P), dtype=np.float16)
    for ipp in range(KK):
        for jpp in range(KK):
            dj = jpp - 3
            mask[ipp * KK + jpp] = ((wcol + dj >= 0) & (wcol + dj < W))
    maskT = np.ascontiguousarray(
        mask.reshape(49, NBLK, 128).transpose(2, 1, 0).reshape(C, NBLK * 49)
    )

    eye = np.zeros((C, 64), dtype=np.float16)
    eye[np.arange(C), np.arange(C) % 64] = 1.0

    in_maps = []
    for core in range(8):
        b, half = divmod(core, 2)
        h0 = HH * half
        # xn: core's own pixels, [C, P] (normal layout, for kernel gen)
        xn = np.ascontiguousarray(
            x[b, :, h0:h0 + HH, :].reshape(C, P).astype(np.float16)
        )
        # shifted transposed copies: xT14[k, (cpy*XBLK + ib)*128 + c]
        # copy cpy=(dj, par): in-copy pixel i (block ib-2, lane k) holds
        # x[b, c, abs_pixel] with abs_pixel = h0*W + (ib-2)*128 + k + (dj-3) + 64*par
        PAD = 6 * W
        xpadT = np.zeros((PAD + H * W + PAD, C), dtype=np.float16)
        xpadT[PAD:PAD + H * W] = x[b].reshape(C, H * W).T.astype(np.float16)
        xt14 = np.zeros((C, 14 * XBLK * 128), dtype=np.float16)
        base0 = PAD + h0 * W
        for dj in range(KK):
            for par in range(2):
                cpy = dj * 2 + par
                st = base0 - 2 * 128 + (dj - 3) + 64 * par
                seg = xpadT[st: st + XBLK * 128]          # [XBLK*128, C]
                # -> [k, ib, c] with partition dim k = pixel-in-block
                seg = np.ascontiguousarray(
                    seg.reshape(XBLK, 128, C).transpose(1, 0, 2)
                )
                xt14[:, cpy * XBLK * 128:(cpy + 1) * XBLK * 128] = (
                    seg.reshape(128, XBLK * 128)
                )
        in_maps.append({
            "xt14": xt14, "xn": xn, "w1sT": w1sT, "b1f": b1fc,
            "w2Ta": w2Ta, "maskT": maskT, "eyec": eye,
        })
    return in_maps


def run(inputs: dict, trace: bool = False):
    from concourse.bass_utils import run_bass_kernel_spmd

    nc = _get_nc()
    in_maps = _host_prep(**inputs)
    res = run_bass_kernel_spmd(
        nc, in_maps, core_ids=list(range(8)), trace=trace,
    )
    out = np.zeros((B, C, H, W), dtype=np.float32)
    for core in range(8):
        b, half = divmod(core, 2)
        h0 = HH * half
        arr = res.results[core]["out"].astype(np.float32)    # [q, blk*128+c]
        arr = arr.reshape(128, NBLK, 128).transpose(1, 0, 2).reshape(P, C)
        out[b, :, h0:h0 + HH, :] = arr.T.reshape(C, HH, W)
    return out, res


def kernel(**inputs) -> np.ndarray:
    out, _ = run(inputs, trace=False)
    return out


# revision 12
# speedup vs baseline: 1.1613x; 1.1613x over previous
# Involution2d (K=7) Trainium2 kernel — 8-core SPMD, batch+spatial sharding.
#
# V3: the multiply-accumulate runs on TensorE as diag-matmuls.
# For each 128-pixel block (2 image rows) the involution is
#   out_T[q, c] = sum_o ker[o, P0+q] * xT[P0 + q + s_o, c]
#               = sum_o matmul(lhsT=diag(kerT_blk[:, o]), rhs=xT_shift_o_blk)
# accumulated across the 49 offsets in fp32 PSUM.  Two tricks make every
# operand a plain aligned tile:
#   - the host ships 14 shifted transposed-x copies (7 column shifts dj x 2
#     row parities); the remaining row shift is then an even multiple of 64
#     = a whole number of 128-pixel blocks, i.e. pure tile indexing.
#   - the diag stationaries use unshifted kerT columns: one fp16 4x-mode
#     tensor_scalar (identity * per-partition scalar) per (offset, block),
#     split between VectorE and ScalarE.
# kerT ([pixel, 49] layout) is generated directly by making the f-slice the
# matmul stationary; b2 rides an appended ones-row, the W-edge mask (which
# also kills every row-wrap artifact of pixel-index shifting) multiplies in
# PSUM.  V2 (145us) was Vector-bound on broadcast-copy/mul/add chains; V3
# replaces all of that with 784 small matmuls + 784 tiny diag builds.
import numpy as np

EPS = 1e-5
KK = 7
C = 128
H = 64
W = 64
B = 4
HH = 32              # rows per core
P = HH * W           # 2048 output pixels per core
NBLK = 16            # 128-pixel blocks per core
XBLK = 19            # blocks per shifted-x copy (offsets -2..+1 around 16)
GEN_CHUNK = 512

_STATE = {}


def _build():
    import concourse.tile as tile
    from concourse import bacc, mybir

    f32 = mybir.dt.float32
    f16 = mybir.dt.float16
    nc = bacc.Bacc("TRN2", target_bir_lowering=False, debug=False)

    # xT14[k, (dj*2+par)*XBLK*128 + blk*128 + c]
    xt_d = nc.dram_tensor("xt14", [C, 14 * XBLK * 128], f16, kind="ExternalInput").ap()
    xn_d = nc.dram_tensor("xn", [C, P], f16, kind="ExternalInput").ap()
    w1sT_d = nc.dram_tensor("w1sT", [C, 32], f16, kind="ExternalInput").ap()
    b1f_d = nc.dram_tensor("b1f", [32, 1], f32, kind="ExternalInput").ap()
    w2Ta_d = nc.dram_tensor("w2Ta", [33, 49], f16, kind="ExternalInput").ap()
    maskT_d = nc.dram_tensor("maskT", [C, NBLK * 49], f16, kind="ExternalInput").ap()
    eye_d = nc.dram_tensor("eyec", [C, 64], f16, kind="ExternalInput").ap()
    out_d = nc.dram_tensor("out", [C, P], f16, kind="ExternalOutput").ap()

    with tile.TileContext(nc) as tc:
        with (
            tc.tile_pool(name="consts", bufs=1) as cpool,
            tc.tile_pool(name="dpool", bufs=6) as dpool,
            tc.tile_pool(name="pgen", bufs=2, space="PSUM") as pgen,
            tc.tile_pool(name="pout", bufs=2, space="PSUM") as pout,
        ):
            xn = cpool.tile([C, P], f16, tag="xn")
            nc.sync.dma_start(xn[:], xn_d)
            w1sT = cpool.tile([C, 32], f16, tag="w1")
            nc.sync.dma_start(w1sT[:], w1sT_d)
            b1f = cpool.tile([32, 1], f32, tag="b1")
            nc.sync.dma_start(b1f[:], b1f_d)
            w2Ta = cpool.tile([33, 49], f16, tag="w2")
            nc.sync.dma_start(w2Ta[:], w2Ta_d)
            maskT = cpool.tile([C, NBLK * 49], f16, tag="mask")
            nc.sync.dma_start(maskT[:], maskT_d)
            eye = cpool.tile([C, 64], f16, tag="eye")
            nc.sync.dma_start(eye[:], eye_d)
            xt = cpool.tile([C, 14 * XBLK * 128], f16, tag="xt")
            nc.sync.dma_start(xt[:], xt_d)

            f_aug = cpool.tile([33, P], f16, tag="f")
            nc.vector.memset(f_aug[32:33, :], 1.0)
            kerT = cpool.tile([C, NBLK * 49], f32, tag="kerT")
            outT = cpool.tile([C, P], f16, tag="outT")

            # ---- kernel generation: f = relu(w1s^T x + b1) ----
            for ci in range(P // GEN_CHUNK):
                sl = slice(ci * GEN_CHUNK, (ci + 1) * GEN_CHUNK)
                f1 = pgen.tile([32, GEN_CHUNK], f32, tag="f1")
                nc.tensor.matmul(f1[:], w1sT[:], xn[:, sl], start=True, stop=True)
                nc.scalar.activation(
                    f_aug[0:32, sl], f1[:], mybir.ActivationFunctionType.Relu,
                    bias=b1f[:],
                )

            # ---- kerT[q, o] per block: (f_blk^T w2 + b2) * maskT ----
            for blk in range(NBLK):
                ksl = slice(blk * 49, (blk + 1) * 49)
                kps = pgen.tile([C, 512], f32, tag="kps")
                nc.tensor.matmul(
                    kps[:, 0:49], f_aug[:, blk * 128:(blk + 1) * 128], w2Ta[:],
                    start=True, stop=True,
                )
                nc.vector.tensor_mul(kerT[:, ksl], kps[:, 0:49], maskT[:, ksl])

            # ---- involution: per block, one batched diag build (compact
            # [k, o, j] sub-diags) + 49 x 4 concurrent 32x32 sub-matmuls ----
            NSE = 11       # diag builds done one-by-one on ScalarE
            NDV = 49 - NSE  # diag builds batched on VectorE
            for blk in range(NBLK):
                ops = pout.tile([C, 512], f32, tag="ops")
                d_all = dpool.tile([C, 49 * 64], f16, tag="dall")
                nc.vector.tensor_mul(
                    d_all[:, 0:NDV * 64].rearrange("k (o j) -> k o j", j=64),
                    eye[:].unsqueeze(1).broadcast_to([C, NDV, 64]),
                    kerT[:, blk * 49: blk * 49 + NDV].unsqueeze(2)
                        .broadcast_to([C, NDV, 64]),
                )
                for o in range(NDV, 49):
                    nc.scalar.activation(
                        d_all[:, o * 64:(o + 1) * 64], eye[:],
                        mybir.ActivationFunctionType.Copy,
                        scale=kerT[:, blk * 49 + o: blk * 49 + o + 1],
                    )
                for o in range(49):
                    di, dj = divmod(o, 7)
                    sh = di - 3
                    par = sh & 1
                    m = (sh - par) >> 1           # in {-2, -1, 0, 1}
                    cpy = dj * 2 + par
                    xoff = (cpy * XBLK + blk + m + 2) * 128
                    for i in range(2):
                        nc.tensor.matmul(
                            ops[64 * i:64 * i + 64, 0:128],
                            d_all[64 * i:64 * i + 64,
                                  o * 64:(o + 1) * 64],
                            xt[64 * i:64 * i + 64, xoff: xoff + 128],
                            start=(o == 0), stop=(o == 48),
                            tile_position=(64 * i, 64 * i),
                        )
                nc.vector.tensor_copy(
                    outT[:, blk * 128:(blk + 1) * 128], ops[:, 0:128]
                )

            nc.sync.dma_start(out_d, outT[:])

    nc.compile()
    return nc


def _get_nc():
    if "nc" not in _STATE:
        _STATE["nc"] = _build()
    return _STATE["nc"]


def _host_prep(x, w1, b1, bn_gamma, bn_beta, bn_mean, bn_var, w2, b2):
    x = np.asarray(x, dtype=np.float32)
    scale = np.asarray(bn_gamma) / np.sqrt(np.asarray(bn_var) + EPS)
    w1s = (np.asarray(w1) * scale[:, None]).astype(np.float32)
    b1f = (np.asarray(b1) * scale + np.asarray(bn_beta)
           - np.asarray(bn_mean) * scale).astype(np.float32)
    w1sT = np.ascontiguousarray(w1s.T.astype(np.float16))        # [128, 32]
    b1fc = np.ascontiguousarray(b1f[:, None].astype(np.float32))
    w2Ta = np.zeros((33, 49), dtype=np.float16)
    w2Ta[:32] = np.asarray(w2, np.float32).T.astype(np.float16)
    w2Ta[32] = np.asarray(b2, np.float32).astype(np.float16)

    # maskT[k, blk*49 + o]: kernel zeroed where w + dj leaves the row
    wcol = np.arange(P, dtype=np.int64) % W
    mask = np.zeros((49, P), dtype=np.float16)
    for ipp in range(KK):
        for jpp in range(KK):
            dj = jpp - 3
            mask[ipp * KK + jpp] = ((wcol + dj >= 0) & (wcol + dj < W))
    maskT = np.ascontiguousarray(
        mask.reshape(49, NBLK, 128).transpose(2, 1, 0).reshape(C, NBLK * 49)
    )

    eye = np.zeros((C, 64), dtype=np.float16)
    eye[np.arange(C), np.arange(C) % 64] = 1.0

    in_maps = []
    for core in range(8):
        b, half = divmod(core, 2)
        h0 = HH * half
        # xn: core's own pixels, [C, P] (normal layout, for kernel gen)
        xn = np.ascontiguousarray(
            x[b, :, h0:h0 + HH, :].reshape(C, P).astype(np.float16)
        )
        # shifted transposed copies: xT14[k, (cpy*XBLK + ib)*128 + c]
        # copy cpy=(dj, par): in-copy pixel i (block ib-2, lane k) holds
        # x[b, c, abs_pixel] with abs_pixel = h0*W + (ib-2)*128 + k + (dj-3) + 64*par
        PAD = 6 * W
        xpadT = np.zeros((PAD + H * W + PAD, C), dtype=np.float16)
        xpadT[PAD:PAD + H * W] = x[b].reshape(C, H * W).T.astype(np.float16)
        xt14 = np.zeros((C, 14 * XBLK * 128), dtype=np.float16)
        base0 = PAD + h0 * W
        for dj in range(KK):
            for par in range(2):
                cpy = dj * 2 + par
                st = base0 - 2 * 128 + (dj - 3) + 64 * par
                seg = xpadT[st: st + XBLK * 128]          # [XBLK*128, C]
                # -> [k, ib, c] with partition dim k = pixel-in-block
                seg = np.ascontiguousarray(
                    seg.reshape(XBLK, 128, C).transpose(1, 0, 2)
                )
                xt14[:, cpy * XBLK * 128:(cpy + 1) * XBLK * 128] = (
                    seg.reshape(128, XBLK * 128)
                )
        in_maps.append({
            "xt14": xt14, "xn": xn, "w1sT": w1sT, "b1f": b1fc,
            "w2Ta": w2Ta, "maskT": maskT, "eyec": eye,
        })
    return in_maps


def run(inputs: dict, trace: bool = False):
    from concourse.bass_utils import run_bass_kernel_spmd

    nc = _get_nc()
    in_maps = _host_prep(**inputs)
    res = run_bass_kernel_spmd(
        nc, in_maps, core_ids=list(range(8)), trace=trace,
    )
    out = np.zeros((B, C, H, W), dtype=np.float32)
    for core in range(8):
        b, half = divmod(core, 2)
        h0 = HH * half
        arr = res.results[core]["out"].astype(np.float32)    # [q, blk*128+c]
        arr = arr.reshape(128, NBLK, 128).transpose(1, 0, 2).reshape(P, C)
        out[b, :, h0:h0 + HH, :] = arr.T.reshape(C, HH, W)
    return out, res


def kernel(**inputs) -> np.ndarray:
    out, _ = run(inputs, trace=False)
    return out
